# revision 43
# baseline (speedup 1.0000x reference)
"""Causal multi-head attention block (QKV proj + RoPE + causal softmax attention
+ output proj) for Trainium2, sharded over 8 NeuronCores.

Problem shapes (hardcoded): B=2, T=2048, DIM=1024, H=16, DH=64.

Sharding: tensor-parallel over heads. Core c owns heads {2c, 2c+1} for BOTH
batches: it computes Q/K/V projections for its 128 head-columns (reading the
full replicated x^T), runs RoPE + causal-softmax attention for its 4 (batch,
head) pairs, then an AllToAll redistributes y^T so each core holds all 1024
y-dims for a 512-token slice and computes that slice of y @ W_out.

All matmuls run as float32r (TF32-like rounded fp32, full PE rate at N>=512);
everything else fp32. Softmax skips the max-subtraction (scores are O(6) for
unit-scale inputs, exp is safe in fp32) and applies causality by multiplying
exp(scores) by a 0/1 triangular mask on the diagonal 128x512 blocks only;
strictly-future blocks are never computed.

b_qkv / b_out handling: b_qkv is structurally zero for this problem (spec fill
"zeros"); if a nonzero b_qkv is ever passed, a bias-enabled program variant is
built instead. b_out is added on the host.
"""

import numpy as np

B = 2
T = 2048
D = 1024
H = 16
DH = 64
NCORES = 8
TT = B * T  # 4096 tokens total
HPC = H // NCORES  # 2 heads per core

_CACHE = {}


def _build(with_collective=True, has_bias=False):
    """Build the SPMD Bass program. Returns (nc, in_names)."""
    import concourse.bass as bass
    import concourse.tile as tile
    from concourse import bacc, mybir, masks
    from contextlib import ExitStack

    F32 = mybir.dt.float32
    F32R = mybir.dt.float32r
    AF = mybir.ActivationFunctionType

    nc = bacc.Bacc("TRN2", target_bir_lowering=False, debug=False,
                   num_devices=NCORES if with_collective else 1)

    # ---- DRAM I/O ----------------------------------------------------------
    xT_d = nc.dram_tensor("xT", [D, TT], F32R, kind="ExternalInput").ap()
    wq_d = nc.dram_tensor("wq", [D, 128], F32R, kind="ExternalInput").ap()
    wk_d = nc.dram_tensor("wk", [D, 128], F32R, kind="ExternalInput").ap()
    wv_d = nc.dram_tensor("wv", [D, 128], F32R, kind="ExternalInput").ap()
    cos_d = nc.dram_tensor("cosT", [128, T], F32, kind="ExternalInput").ap()
    sin_d = nc.dram_tensor("sinN", [128, T], F32, kind="ExternalInput").ap()
    ones_d = nc.dram_tensor("ones16", [128, 16], F32R, kind="ExternalInput").ap()
    perm_d = nc.dram_tensor("perm32", [128, 128], F32R, kind="ExternalInput").ap()
    wout_d = nc.dram_tensor("wout", [D, D], F32R, kind="ExternalInput").ap()
    out_d = nc.dram_tensor("out", [TT // NCORES, D], F32, kind="ExternalOutput").ap()
    if has_bias:
        bq_d = nc.dram_tensor("bqkv", [128, 3], F32, kind="ExternalInput").ap()

    a2a_in = nc.dram_tensor("a2a_in", [NCORES, 128, TT // NCORES], F32R,
                            kind="Internal").ap()
    a2a_out = nc.dram_tensor("a2a_out", [NCORES, 128, TT // NCORES], F32R,
                             kind="Internal").ap()

    with tile.TileContext(nc) as tc:
        with ExitStack() as ctx:
            const = ctx.enter_context(tc.tile_pool(name="const", bufs=1))
            xtp = ctx.enter_context(tc.tile_pool(name="xtp", bufs=4))
            qkp = ctx.enter_context(tc.tile_pool(name="qkp", bufs=2))
            vtp = ctx.enter_context(tc.tile_pool(name="vtp", bufs=2))
            ybigp = ctx.enter_context(tc.tile_pool(name="ybigp", bufs=1))
            ropep = ctx.enter_context(tc.tile_pool(name="ropep", bufs=2))
            pp = ctx.enter_context(tc.tile_pool(name="pp", bufs=5))
            op = ctx.enter_context(tc.tile_pool(name="op", bufs=1))
            smallp = ctx.enter_context(tc.tile_pool(name="smallp", bufs=2))
            ps_s = ctx.enter_context(tc.tile_pool(name="ps_s", bufs=2, space="PSUM"))
            ps_y = ctx.enter_context(tc.tile_pool(name="ps_y", bufs=4, space="PSUM"))

            # ---- constants -------------------------------------------------
            wq_sb = const.tile([128, 1024], F32R, tag="wq")
            wk_sb = const.tile([128, 1024], F32R, tag="wk")
            wv_sb = const.tile([128, 1024], F32R, tag="wv")
            for wsb, wd in ((wq_sb, wq_d), (wk_sb, wk_d), (wv_sb, wv_d)):
                wdr = wd.rearrange("(c p) m -> p c m", p=128)
                nc.sync.dma_start(wsb[:], wdr[:])
            cos_sb = const.tile([128, T], F32, tag="cos")
            sin_sb = const.tile([128, T], F32, tag="sin")
            nc.sync.dma_start(cos_sb[:], cos_d[:])
            nc.sync.dma_start(sin_sb[:], sin_d[:])
            ident = const.tile([128, 128], F32, tag="ident")
            masks.make_identity(nc, ident[:])
            perm_sb = const.tile([128, 128], F32R, tag="perm")
            nc.sync.dma_start(perm_sb[:], perm_d[:])
            if has_bias:
                bq_sb = const.tile([128, 3], F32, tag="bq")
                nc.sync.dma_start(bq_sb[:], bq_d[:])

            ytile = ybigp.tile([128, TT], F32R, tag="ybig")  # y^T accumulator

            TC = T // 512  # 4 token chunks per batch

            QTs, KTs, vhs = [], [], []
            for b in range(B):
                # ---- QKV projection + RoPE for batch b ---------------------
                QT = qkp.tile([128, T], F32R, tag="qt")
                KT = qkp.tile([128, T], F32R, tag="kt")
                VT = qkp.tile([128, T], F32R, tag="vt", bufs=1)
                for tcx in range(TC):
                    col0 = b * T + tcx * 512
                    ps_qk = ps_s.tile([128, 1024], F32, tag="s")  # q | k
                    ps_v = ps_y.tile([128, 512], F32, tag="y", name="ps_v")
                    xTr = xT_d.rearrange("(c p) t -> p c t", p=128)
                    for kc4 in range(4):  # one DMA brings 2 contraction chunks
                        xt = xtp.tile([128, 2 * 512], F32R, tag="xt")
                        nc.sync.dma_start(
                            xt[:], xTr[:, 2 * kc4:2 * kc4 + 2, col0:col0 + 512])
                        for kcs in range(2):
                            kc = 2 * kc4 + kcs
                            st = (kc == 0)
                            sp = (kc == 7)
                            xts = xt[:, kcs * 512:(kcs + 1) * 512]
                            nc.tensor.matmul(ps_qk[:, 0:512], wq_sb[:, bass.ts(kc, 128)],
                                             xts, start=st, stop=sp)
                            nc.tensor.matmul(ps_qk[:, 512:1024], wk_sb[:, bass.ts(kc, 128)],
                                             xts, start=st, stop=sp)
                            nc.tensor.matmul(ps_v[:], wv_sb[:, bass.ts(kc, 128)],
                                             xts, start=st, stop=sp)
                    # V: plain copy psum -> VT (f32r), with b_v folded in if present
                    if has_bias:
                        nc.vector.tensor_scalar_add(
                            VT[:, tcx * 512:(tcx + 1) * 512], ps_v[:],
                            bq_sb[:, 2:3])
                    else:
                        nc.vector.tensor_copy(VT[:, tcx * 512:(tcx + 1) * 512],
                                              ps_v[:])
                    # RoPE for Q and K: copy psum -> sbuf (ACT, f32r), build the
                    # 32-block-swapped copy on the PE (perm matmul -> psum),
                    # then 4 muls + 2 adds on the DVE (tensor_tensor is 1x
                    # either way, so reading the swap from PSUM costs nothing).
                    qraw = ropep.tile([128, 1024], F32R, tag="qraw")
                    if has_bias:
                        nc.scalar.activation(qraw[:, 0:512], ps_qk[:, 0:512],
                                             AF.Identity, bias=bq_sb[:, 0:1])
                        nc.scalar.activation(qraw[:, 512:1024], ps_qk[:, 512:1024],
                                             AF.Identity, bias=bq_sb[:, 1:2])
                    else:
                        nc.scalar.activation(qraw[:], ps_qk[:], AF.Identity)
                    ps_sw = [ps_y.tile([128, 512], F32, tag="y", name=f"ps_sw{_i}")
                             for _i in range(2)]
                    for _i in range(2):
                        nc.tensor.matmul(ps_sw[_i][:], perm_sb[:],
                                         qraw[:, _i * 512:(_i + 1) * 512],
                                         start=True, stop=True)
                    tcols = slice(tcx * 512, (tcx + 1) * 512)
                    nc.vector.tensor_mul(qraw[:, 0:512], qraw[:, 0:512], cos_sb[:, tcols])
                    nc.vector.tensor_mul(qraw[:, 512:1024], qraw[:, 512:1024],
                                         cos_sb[:, tcols])
                    t2 = ropep.tile([128, 1024], F32, tag="t2")
                    nc.vector.tensor_mul(t2[:, 0:512], ps_sw[0][:], sin_sb[:, tcols])
                    nc.vector.tensor_mul(t2[:, 512:1024], ps_sw[1][:], sin_sb[:, tcols])
                    nc.vector.tensor_add(QT[:, tcols], qraw[:, 0:512], t2[:, 0:512])
                    nc.vector.tensor_add(KT[:, tcols], qraw[:, 512:1024], t2[:, 512:1024])

                # ---- V transpose into per-head augmented buffers -----------
                vh = [vtp.tile([128, 16 * 65], F32R, tag=f"vh{h}", name=f"vh{h}")
                      for h in range(2)]
                QTs.append(QT)
                KTs.append(KT)
                vhs.append(vh)
                for h in range(2):
                    ones_cols = vh[h][:].rearrange("p (n c) -> p n c", c=65)[:, :, 64]
                    nc.sync.dma_start(ones_cols, ones_d[:])
                for kb in range(16):
                    ps_t = ps_y.tile([128, 512], F32, tag="y", name="ps_t")
                    nc.tensor.transpose(ps_t[:, 0:128],
                                        VT[:, kb * 128:(kb + 1) * 128].bitcast(F32),
                                        ident[:])
                    for h in range(2):
                        nc.vector.tensor_copy(vh[h][:, kb * 65:kb * 65 + 64],
                                              ps_t[:, h * 64:h * 64 + 64])

            # ---- load W_out early: overlaps with attention compute ---------
            wout_sb = const.tile([128, 8 * 1024], F32R, tag="wout")
            wor = wout_d.rearrange("(c p) m -> p c m", p=128)
            nc.sync.dma_start(wout_sb[:], wor[:])

            for b in range(B):
                QT, KT, vh = QTs[b], KTs[b], vhs[b]
                # ---- attention for batch b ---------------------------------
                for qc in range(TC):
                    nkb = 4 * (qc + 1)  # causal: k chunks of 128 needed
                    qcols = slice(qc * 512, (qc + 1) * 512)
                    ps_yh = [ps_y.tile([128, 512], F32, tag="y", name=f"ps_y{_h}")
                             for _h in range(2)]
                    for kb in range(nkb):
                        # Diagonal k-chunks: q-columns < r are fully masked, so
                        # S / exp / PV all shrink to the valid span [r:512].
                        r = max(kb * 128 - qc * 512, 0)
                        nv = 512 - r  # valid q columns
                        ps_st = ps_s.tile([128, 1024], F32, tag="s")
                        for h in range(2):
                            nc.tensor.matmul(
                                ps_st[:, h * 512 + r:(h + 1) * 512],
                                KT[h * 64:(h + 1) * 64, kb * 128:(kb + 1) * 128],
                                QT[h * 64:(h + 1) * 64,
                                   qc * 512 + r:(qc + 1) * 512],
                                start=True, stop=True)
                        p = pp.tile([128, 1024], F32R, tag="p")
                        if r == 0:
                            nc.scalar.activation(p[:], ps_st[:], AF.Exp,
                                                 scale=float(DH) ** -0.5)
                        else:
                            for h in range(2):
                                nc.scalar.activation(
                                    p[:, h * 512 + r:(h + 1) * 512],
                                    ps_st[:, h * 512 + r:(h + 1) * 512],
                                    AF.Exp, scale=float(DH) ** -0.5)
                        if kb * 128 - qc * 512 >= 0:
                            # Zero strictly-future scores. Only the first 128
                            # valid columns can intersect the triangle (k-row
                            # p <= 127 < f for all later columns).
                            for h in range(2):
                                ph = p[:, h * 512 + r:h * 512 + r + 128]
                                nc.gpsimd.affine_select(
                                    out=ph, in_=ph, pattern=[[1, 128]],
                                    compare_op=mybir.AluOpType.is_ge,
                                    fill=0.0, base=0, channel_multiplier=-1)
                        for h in range(2):
                            nc.tensor.matmul(ps_yh[h][0:65, r:512],
                                             vh[h][:, kb * 65:(kb + 1) * 65],
                                             p[:, h * 512 + r:(h + 1) * 512],
                                             start=(kb == 0), stop=(kb == nkb - 1))
                    # normalize: y / denom (denom = psum row 64)
                    for h in range(2):
                        rr = smallp.tile([1, 512], F32, tag="r")
                        nc.vector.reciprocal(rr[:], ps_yh[h][64:65, :])
                        rb = smallp.tile([64, 512], F32, tag="rb")
                        nc.gpsimd.partition_broadcast(rb[:], rr[:], channels=64)
                        nc.vector.tensor_mul(
                            ytile[h * 64:(h + 1) * 64, b * T + qc * 512: b * T + (qc + 1) * 512],
                            ps_yh[h][0:64, :], rb[:])
                    jslice = 4 * b + qc
                    nc.sync.dma_start(a2a_in[jslice],
                                      ytile[:, jslice * 512:(jslice + 1) * 512])

            # ---- exchange y^T: all-to-all over token slices ----------------
            if with_collective:
                nc.gpsimd.collective_compute(
                    "AllToAll", mybir.AluOpType.bypass,
                    replica_groups=[list(range(NCORES))],
                    ins=[a2a_in[:]], outs=[a2a_out[:]])
            else:  # single-core timing/sim variant: fake the exchange
                for j in range(NCORES):
                    nc.sync.dma_start(a2a_out[j], a2a_in[j])
            yall = ybigp.tile([128, TT], F32R, tag="ybig")
            for i in range(NCORES):
                nc.sync.dma_start(yall[:, i * 512:(i + 1) * 512], a2a_out[i])

            # ---- output projection for my 512-token slice ------------------
            for tc4 in range(4):
                ps_o = ps_s.tile([128, 1024], F32, tag="s")
                for oc in range(2):
                    for i in range(8):
                        nc.tensor.matmul(
                            ps_o[:, oc * 512:(oc + 1) * 512],
                            yall[:, i * 512 + tc4 * 128: i * 512 + (tc4 + 1) * 128],
                            wout_sb[:, i * 1024 + oc * 512: i * 1024 + (oc + 1) * 512],
                            start=(i == 0), stop=(i == 7))
                o_sb = op.tile([128, 1024], F32, tag="o")
                nc.scalar.activation(o_sb[:], ps_o[:], AF.Identity)
                nc.sync.dma_start(out_d[tc4 * 128:(tc4 + 1) * 128, :], o_sb[:])

    nc.compile()
    return nc


def _host_tables():
    pos = np.arange(T, dtype=np.float32)
    idx = np.arange(32, dtype=np.float32)
    inv = (10000.0 ** (-2.0 * idx / 64.0)).astype(np.float32)
    ang = (inv[:, None] * pos[None, :]).astype(np.float32)  # [32, T]
    cos32 = np.cos(ang).astype(np.float32)
    sin32 = np.sin(ang).astype(np.float32)
    cosT = np.tile(cos32, (4, 1))  # [128, T]
    sinN = np.concatenate([-sin32, sin32, -sin32, sin32], axis=0)  # [128, T]
    # causal 0/1 masks for the 4 diagonal offsets r = 0,128,256,384
    ones16 = np.ones((128, 16), dtype=np.float32)
    perm32 = np.zeros((128, 128), dtype=np.float32)
    for j in range(128):
        blk, off = divmod(j, 32)
        perm32[(blk ^ 1) * 32 + off, j] = 1.0
    return cosT, sinN, ones16, perm32


def kernel(x, mask, W_qkv, b_qkv, W_out, b_out):
    from concourse.bass_utils import run_bass_kernel_spmd

    x = np.asarray(x, dtype=np.float32)
    W_qkv = np.asarray(W_qkv, dtype=np.float32)
    b_qkv = np.asarray(b_qkv, dtype=np.float32)
    W_out = np.asarray(W_out, dtype=np.float32)
    b_out = np.asarray(b_out, dtype=np.float32)

    has_bias = bool(np.any(b_qkv != 0.0))
    key = ("main", has_bias)
    if key not in _CACHE:
        _CACHE[key] = _build(with_collective=True, has_bias=has_bias)
    nc = _CACHE[key]

    X = x.reshape(TT, D)
    xT = np.ascontiguousarray(X.T)  # [D, TT]
    cosT, sinN, ones16, perm32 = _host_tables()
    Wq, Wk, Wv = W_qkv[:, 0:D], W_qkv[:, D:2 * D], W_qkv[:, 2 * D:3 * D]
    wout = np.ascontiguousarray(W_out)

    in_maps = []
    for c in range(NCORES):
        hA, hB = 2 * c, 2 * c + 1
        sl = np.s_[:, hA * DH:(hA + 1) * DH], np.s_[:, hB * DH:(hB + 1) * DH]
        m = {
            "xT": xT,
            "wq": np.ascontiguousarray(np.concatenate([Wq[sl[0]], Wq[sl[1]]], axis=1)),
            "wk": np.ascontiguousarray(np.concatenate([Wk[sl[0]], Wk[sl[1]]], axis=1)),
            "wv": np.ascontiguousarray(np.concatenate([Wv[sl[0]], Wv[sl[1]]], axis=1)),
            "cosT": cosT, "sinN": sinN, "ones16": ones16, "perm32": perm32,
            "wout": wout,
        }
        if has_bias:
            bq = np.stack([
                np.concatenate([b_qkv[hA * DH:(hA + 1) * DH], b_qkv[hB * DH:(hB + 1) * DH]]),
                np.concatenate([b_qkv[D + hA * DH:D + (hA + 1) * DH], b_qkv[D + hB * DH:D + (hB + 1) * DH]]),
                np.concatenate([b_qkv[2 * D + hA * DH:2 * D + (hA + 1) * DH], b_qkv[2 * D + hB * DH:2 * D + (hB + 1) * DH]]),
            ], axis=1).astype(np.float32)  # [128, 3]
            m["bqkv"] = bq
        in_maps.append(m)

    res = run_bass_kernel_spmd(nc, in_maps, core_ids=list(range(NCORES)))
    out = np.concatenate([res.results[c]["out"] for c in range(NCORES)], axis=0)
    out = out + b_out[None, :]
    return out.reshape(B, T, D).astype(np.float32)


if __name__ == "__main__":
    # quick self-check against a numpy reference
    rng = np.random.default_rng(0)
    x = rng.standard_normal((B, T, D)).astype(np.float32)
    mask = np.tril(np.ones((T, T), dtype=np.int32))[None, None]
    W_qkv = (rng.standard_normal((D, 3 * D)) * D ** -0.5).astype(np.float32)
    b_qkv = np.zeros(3 * D, np.float32)
    W_out = (rng.standard_normal((D, D)) * D ** -0.5).astype(np.float32)
    b_out = np.zeros(D, np.float32)

    def ref_np(x):
        q = x.reshape(TT, D) @ W_qkv[:, :D] + b_qkv[:D]
        k = x.reshape(TT, D) @ W_qkv[:, D:2 * D] + b_qkv[D:2 * D]
        v = x.reshape(TT, D) @ W_qkv[:, 2 * D:] + b_qkv[2 * D:]
        def heads(t):
            return t.reshape(B, T, H, DH).transpose(0, 2, 1, 3)
        q, k, v = heads(q), heads(k), heads(v)
        half = DH // 2
        idx = np.arange(half, dtype=np.float32)
        posn = np.arange(T, dtype=np.float32)[:, None]
        invf = 10000.0 ** (-2.0 * idx / (2.0 * half))
        ang = posn * invf
        cosv, sinv = np.cos(ang), np.sin(ang)
        def rot(t):
            a, b2 = t[..., :half], t[..., half:]
            return np.concatenate([a * cosv - b2 * sinv, a * sinv + b2 * cosv], -1)
        q, k = rot(q), rot(k)
        s = np.einsum("bhqd,bhkd->bhqk", q, k) / np.sqrt(DH)
        cm = np.tril(np.ones((T, T))) == 0
        s = np.where(cm[None, None], -np.inf, s)
        p = np.exp(s - s.max(-1, keepdims=True))
        p = p / p.sum(-1, keepdims=True)
        y = np.einsum("bhqk,bhkd->bhqd", p, v)
        y = y.transpose(0, 2, 1, 3).reshape(B, T, D)
        return y @ W_out + b_out

    got = kernel(x, mask, W_qkv, b_qkv, W_out, b_out)
    want = ref_np(x.astype(np.float64))
    err = np.abs(got - want).max()
    print(f"maxabs err vs np-f64 ref: {err:.3e}  (scale {np.abs(want).max():.3f})")


# revision 44
# speedup vs baseline: 1.0028x; 1.0028x over previous
"""Causal multi-head attention block (QKV proj + RoPE + causal softmax attention
+ output proj) for Trainium2, sharded over 8 NeuronCores.

Problem shapes (hardcoded): B=2, T=2048, DIM=1024, H=16, DH=64.

Sharding: tensor-parallel over heads. Core c owns heads {2c, 2c+1} for BOTH
batches: it computes Q/K/V projections for its 128 head-columns (reading the
full replicated x^T), runs RoPE + causal-softmax attention for its 4 (batch,
head) pairs, then an AllToAll redistributes y^T so each core holds all 1024
y-dims for a 512-token slice and computes that slice of y @ W_out.

All matmuls run as float32r (TF32-like rounded fp32, full PE rate at N>=512);
everything else fp32. Softmax skips the max-subtraction (scores are O(6) for
unit-scale inputs, exp is safe in fp32) and applies causality by multiplying
exp(scores) by a 0/1 triangular mask on the diagonal 128x512 blocks only;
strictly-future blocks are never computed.

b_qkv / b_out handling: b_qkv is structurally zero for this problem (spec fill
"zeros"); if a nonzero b_qkv is ever passed, a bias-enabled program variant is
built instead. b_out is added on the host.
"""

import numpy as np

B = 2
T = 2048
D = 1024
H = 16
DH = 64
NCORES = 8
TT = B * T  # 4096 tokens total
HPC = H // NCORES  # 2 heads per core

_CACHE = {}


def _build(with_collective=True, has_bias=False):
    """Build the SPMD Bass program. Returns (nc, in_names)."""
    import concourse.bass as bass
    import concourse.tile as tile
    from concourse import bacc, mybir, masks
    from contextlib import ExitStack

    F32 = mybir.dt.float32
    F32R = mybir.dt.float32r
    AF = mybir.ActivationFunctionType

    nc = bacc.Bacc("TRN2", target_bir_lowering=False, debug=False,
                   num_devices=NCORES if with_collective else 1)

    # ---- DRAM I/O ----------------------------------------------------------
    xT_d = nc.dram_tensor("xT", [D, TT], F32R, kind="ExternalInput").ap()
    wq_d = nc.dram_tensor("wq", [D, 128], F32R, kind="ExternalInput").ap()
    wk_d = nc.dram_tensor("wk", [D, 128], F32R, kind="ExternalInput").ap()
    wv_d = nc.dram_tensor("wv", [D, 128], F32R, kind="ExternalInput").ap()
    cos_d = nc.dram_tensor("cosT", [128, T], F32, kind="ExternalInput").ap()
    sin_d = nc.dram_tensor("sinN", [128, T], F32, kind="ExternalInput").ap()
    ones_d = nc.dram_tensor("ones16", [128, 16], F32R, kind="ExternalInput").ap()
    perm_d = nc.dram_tensor("perm32", [128, 128], F32R, kind="ExternalInput").ap()
    wout_d = nc.dram_tensor("wout", [D, D], F32R, kind="ExternalInput").ap()
    out_d = nc.dram_tensor("out", [TT // NCORES, D], F32, kind="ExternalOutput").ap()
    if has_bias:
        bq_d = nc.dram_tensor("bqkv", [128, 3], F32, kind="ExternalInput").ap()

    a2a_in = nc.dram_tensor("a2a_in", [NCORES, 128, TT // NCORES], F32R,
                            kind="Internal").ap()
    a2a_out = nc.dram_tensor("a2a_out", [NCORES, 128, TT // NCORES], F32R,
                             kind="Internal").ap()

    with tile.TileContext(nc) as tc:
        with ExitStack() as ctx:
            const = ctx.enter_context(tc.tile_pool(name="const", bufs=1))
            xtp = ctx.enter_context(tc.tile_pool(name="xtp", bufs=4))
            qkp = ctx.enter_context(tc.tile_pool(name="qkp", bufs=2))
            vtp = ctx.enter_context(tc.tile_pool(name="vtp", bufs=2))
            ybigp = ctx.enter_context(tc.tile_pool(name="ybigp", bufs=1))
            ropep = ctx.enter_context(tc.tile_pool(name="ropep", bufs=2))
            pp = ctx.enter_context(tc.tile_pool(name="pp", bufs=5))
            op = ctx.enter_context(tc.tile_pool(name="op", bufs=1))
            smallp = ctx.enter_context(tc.tile_pool(name="smallp", bufs=2))
            ps_s = ctx.enter_context(tc.tile_pool(name="ps_s", bufs=2, space="PSUM"))
            ps_y = ctx.enter_context(tc.tile_pool(name="ps_y", bufs=4, space="PSUM"))

            # ---- constants -------------------------------------------------
            wq_sb = const.tile([128, 1024], F32R, tag="wq")
            wk_sb = const.tile([128, 1024], F32R, tag="wk")
            wv_sb = const.tile([128, 1024], F32R, tag="wv")
            for wsb, wd in ((wq_sb, wq_d), (wk_sb, wk_d), (wv_sb, wv_d)):
                wdr = wd.rearrange("(c p) m -> p c m", p=128)
                nc.sync.dma_start(wsb[:], wdr[:])
            cos_sb = const.tile([128, T], F32, tag="cos")
            sin_sb = const.tile([128, T], F32, tag="sin")
            nc.sync.dma_start(cos_sb[:], cos_d[:])
            nc.sync.dma_start(sin_sb[:], sin_d[:])
            ident = const.tile([128, 128], F32, tag="ident")
            masks.make_identity(nc, ident[:])
            perm_sb = const.tile([128, 128], F32R, tag="perm")
            nc.sync.dma_start(perm_sb[:], perm_d[:])
            if has_bias:
                bq_sb = const.tile([128, 3], F32, tag="bq")
                nc.sync.dma_start(bq_sb[:], bq_d[:])

            ytile = ybigp.tile([128, TT], F32R, tag="ybig")  # y^T accumulator

            TC = T // 512  # 4 token chunks per batch

            QTs, KTs, vhs = [], [], []
            for b in range(B):
                # ---- QKV projection + RoPE for batch b ---------------------
                QT = qkp.tile([128, T], F32R, tag="qt")
                KT = qkp.tile([128, T], F32R, tag="kt")
                VT = qkp.tile([128, T], F32R, tag="vt", bufs=1)
                for tcx in range(TC):
                    col0 = b * T + tcx * 512
                    ps_qk = ps_s.tile([128, 1024], F32, tag="s")  # q | k
                    ps_v = ps_y.tile([128, 512], F32, tag="y", name="ps_v")
                    xTr = xT_d.rearrange("(c p) t -> p c t", p=128)
                    for kc4 in range(4):  # one DMA brings 2 contraction chunks
                        xt = xtp.tile([128, 2 * 512], F32R, tag="xt")
                        nc.sync.dma_start(
                            xt[:], xTr[:, 2 * kc4:2 * kc4 + 2, col0:col0 + 512])
                        for kcs in range(2):
                            kc = 2 * kc4 + kcs
                            st = (kc == 0)
                            sp = (kc == 7)
                            xts = xt[:, kcs * 512:(kcs + 1) * 512]
                            nc.tensor.matmul(ps_qk[:, 0:512], wq_sb[:, bass.ts(kc, 128)],
                                             xts, start=st, stop=sp)
                            nc.tensor.matmul(ps_qk[:, 512:1024], wk_sb[:, bass.ts(kc, 128)],
                                             xts, start=st, stop=sp)
                            nc.tensor.matmul(ps_v[:], wv_sb[:, bass.ts(kc, 128)],
                                             xts, start=st, stop=sp)
                    # V: plain copy psum -> VT (f32r), with b_v folded in if present
                    if has_bias:
                        nc.vector.tensor_scalar_add(
                            VT[:, tcx * 512:(tcx + 1) * 512], ps_v[:],
                            bq_sb[:, 2:3])
                    else:
                        nc.vector.tensor_copy(VT[:, tcx * 512:(tcx + 1) * 512],
                                              ps_v[:])
                    # RoPE for Q and K: copy psum -> sbuf (ACT, f32r), build the
                    # 32-block-swapped copy on the PE (perm matmul -> psum),
                    # then 4 muls + 2 adds on the DVE (tensor_tensor is 1x
                    # either way, so reading the swap from PSUM costs nothing).
                    qraw = ropep.tile([128, 1024], F32R, tag="qraw")
                    if has_bias:
                        nc.scalar.activation(qraw[:, 0:512], ps_qk[:, 0:512],
                                             AF.Identity, bias=bq_sb[:, 0:1])
                        nc.scalar.activation(qraw[:, 512:1024], ps_qk[:, 512:1024],
                                             AF.Identity, bias=bq_sb[:, 1:2])
                    else:
                        nc.scalar.activation(qraw[:], ps_qk[:], AF.Identity)
                    ps_sw = [ps_y.tile([128, 512], F32, tag="y", name=f"ps_sw{_i}")
                             for _i in range(2)]
                    for _i in range(2):
                        nc.tensor.matmul(ps_sw[_i][:], perm_sb[:],
                                         qraw[:, _i * 512:(_i + 1) * 512],
                                         start=True, stop=True)
                    tcols = slice(tcx * 512, (tcx + 1) * 512)
                    nc.gpsimd.tensor_mul(qraw[:, 0:512], qraw[:, 0:512], cos_sb[:, tcols])
                    nc.gpsimd.tensor_mul(qraw[:, 512:1024], qraw[:, 512:1024],
                                         cos_sb[:, tcols])
                    t2 = ropep.tile([128, 1024], F32, tag="t2")
                    nc.vector.tensor_mul(t2[:, 0:512], ps_sw[0][:], sin_sb[:, tcols])
                    nc.vector.tensor_mul(t2[:, 512:1024], ps_sw[1][:], sin_sb[:, tcols])
                    nc.vector.tensor_add(QT[:, tcols], qraw[:, 0:512], t2[:, 0:512])
                    nc.vector.tensor_add(KT[:, tcols], qraw[:, 512:1024], t2[:, 512:1024])

                # ---- V transpose into per-head augmented buffers -----------
                vh = [vtp.tile([128, 16 * 65], F32R, tag=f"vh{h}", name=f"vh{h}")
                      for h in range(2)]
                QTs.append(QT)
                KTs.append(KT)
                vhs.append(vh)
                for h in range(2):
                    ones_cols = vh[h][:].rearrange("p (n c) -> p n c", c=65)[:, :, 64]
                    nc.sync.dma_start(ones_cols, ones_d[:])
                for kb in range(16):
                    ps_t = ps_y.tile([128, 512], F32, tag="y", name="ps_t")
                    nc.tensor.transpose(ps_t[:, 0:128],
                                        VT[:, kb * 128:(kb + 1) * 128].bitcast(F32),
                                        ident[:])
                    for h in range(2):
                        nc.vector.tensor_copy(vh[h][:, kb * 65:kb * 65 + 64],
                                              ps_t[:, h * 64:h * 64 + 64])

            # ---- load W_out early: overlaps with attention compute ---------
            wout_sb = const.tile([128, 8 * 1024], F32R, tag="wout")
            wor = wout_d.rearrange("(c p) m -> p c m", p=128)
            nc.sync.dma_start(wout_sb[:], wor[:])

            for b in range(B):
                QT, KT, vh = QTs[b], KTs[b], vhs[b]
                # ---- attention for batch b ---------------------------------
                for qc in range(TC):
                    nkb = 4 * (qc + 1)  # causal: k chunks of 128 needed
                    qcols = slice(qc * 512, (qc + 1) * 512)
                    ps_yh = [ps_y.tile([128, 512], F32, tag="y", name=f"ps_y{_h}")
                             for _h in range(2)]
                    for kb in range(nkb):
                        # Diagonal k-chunks: q-columns < r are fully masked, so
                        # S / exp / PV all shrink to the valid span [r:512].
                        r = max(kb * 128 - qc * 512, 0)
                        nv = 512 - r  # valid q columns
                        ps_st = ps_s.tile([128, 1024], F32, tag="s")
                        for h in range(2):
                            nc.tensor.matmul(
                                ps_st[:, h * 512 + r:(h + 1) * 512],
                                KT[h * 64:(h + 1) * 64, kb * 128:(kb + 1) * 128],
                                QT[h * 64:(h + 1) * 64,
                                   qc * 512 + r:(qc + 1) * 512],
                                start=True, stop=True)
                        p = pp.tile([128, 1024], F32R, tag="p")
                        if r == 0:
                            nc.scalar.activation(p[:], ps_st[:], AF.Exp,
                                                 scale=float(DH) ** -0.5)
                        else:
                            for h in range(2):
                                nc.scalar.activation(
                                    p[:, h * 512 + r:(h + 1) * 512],
                                    ps_st[:, h * 512 + r:(h + 1) * 512],
                                    AF.Exp, scale=float(DH) ** -0.5)
                        if kb * 128 - qc * 512 >= 0:
                            # Zero strictly-future scores. Only the first 128
                            # valid columns can intersect the triangle (k-row
                            # p <= 127 < f for all later columns).
                            for h in range(2):
                                ph = p[:, h * 512 + r:h * 512 + r + 128]
                                nc.gpsimd.affine_select(
                                    out=ph, in_=ph, pattern=[[1, 128]],
                                    compare_op=mybir.AluOpType.is_ge,
                                    fill=0.0, base=0, channel_multiplier=-1)
                        for h in range(2):
                            nc.tensor.matmul(ps_yh[h][0:65, r:512],
                                             vh[h][:, kb * 65:(kb + 1) * 65],
                                             p[:, h * 512 + r:(h + 1) * 512],
                                             start=(kb == 0), stop=(kb == nkb - 1))
                    # normalize: y / denom (denom = psum row 64)
                    for h in range(2):
                        rr = smallp.tile([1, 512], F32, tag="r")
                        nc.vector.reciprocal(rr[:], ps_yh[h][64:65, :])
                        rb = smallp.tile([64, 512], F32, tag="rb")
                        nc.gpsimd.partition_broadcast(rb[:], rr[:], channels=64)
                        nc.vector.tensor_mul(
                            ytile[h * 64:(h + 1) * 64, b * T + qc * 512: b * T + (qc + 1) * 512],
                            ps_yh[h][0:64, :], rb[:])
                    jslice = 4 * b + qc
                    nc.sync.dma_start(a2a_in[jslice],
                                      ytile[:, jslice * 512:(jslice + 1) * 512])

            # ---- exchange y^T: all-to-all over token slices ----------------
            if with_collective:
                nc.gpsimd.collective_compute(
                    "AllToAll", mybir.AluOpType.bypass,
                    replica_groups=[list(range(NCORES))],
                    ins=[a2a_in[:]], outs=[a2a_out[:]])
            else:  # single-core timing/sim variant: fake the exchange
                for j in range(NCORES):
                    nc.sync.dma_start(a2a_out[j], a2a_in[j])
            yall = ybigp.tile([128, TT], F32R, tag="ybig")
            for i in range(NCORES):
                nc.sync.dma_start(yall[:, i * 512:(i + 1) * 512], a2a_out[i])

            # ---- output projection for my 512-token slice ------------------
            for tc4 in range(4):
                ps_o = ps_s.tile([128, 1024], F32, tag="s")
                for oc in range(2):
                    for i in range(8):
                        nc.tensor.matmul(
                            ps_o[:, oc * 512:(oc + 1) * 512],
                            yall[:, i * 512 + tc4 * 128: i * 512 + (tc4 + 1) * 128],
                            wout_sb[:, i * 1024 + oc * 512: i * 1024 + (oc + 1) * 512],
                            start=(i == 0), stop=(i == 7))
                o_sb = op.tile([128, 1024], F32, tag="o")
                nc.scalar.activation(o_sb[:], ps_o[:], AF.Identity)
                nc.sync.dma_start(out_d[tc4 * 128:(tc4 + 1) * 128, :], o_sb[:])

    nc.compile()
    return nc


def _host_tables():
    pos = np.arange(T, dtype=np.float32)
    idx = np.arange(32, dtype=np.float32)
    inv = (10000.0 ** (-2.0 * idx / 64.0)).astype(np.float32)
    ang = (inv[:, None] * pos[None, :]).astype(np.float32)  # [32, T]
    cos32 = np.cos(ang).astype(np.float32)
    sin32 = np.sin(ang).astype(np.float32)
    cosT = np.tile(cos32, (4, 1))  # [128, T]
    sinN = np.concatenate([-sin32, sin32, -sin32, sin32], axis=0)  # [128, T]
    # causal 0/1 masks for the 4 diagonal offsets r = 0,128,256,384
    ones16 = np.ones((128, 16), dtype=np.float32)
    perm32 = np.zeros((128, 128), dtype=np.float32)
    for j in range(128):
        blk, off = divmod(j, 32)
        perm32[(blk ^ 1) * 32 + off, j] = 1.0
    return cosT, sinN, ones16, perm32


def kernel(x, mask, W_qkv, b_qkv, W_out, b_out):
    from concourse.bass_utils import run_bass_kernel_spmd

    x = np.asarray(x, dtype=np.float32)
    W_qkv = np.asarray(W_qkv, dtype=np.float32)
    b_qkv = np.asarray(b_qkv, dtype=np.float32)
    W_out = np.asarray(W_out, dtype=np.float32)
    b_out = np.asarray(b_out, dtype=np.float32)

    has_bias = bool(np.any(b_qkv != 0.0))
    key = ("main", has_bias)
    if key not in _CACHE:
        _CACHE[key] = _build(with_collective=True, has_bias=has_bias)
    nc = _CACHE[key]

    X = x.reshape(TT, D)
    xT = np.ascontiguousarray(X.T)  # [D, TT]
    cosT, sinN, ones16, perm32 = _host_tables()
    Wq, Wk, Wv = W_qkv[:, 0:D], W_qkv[:, D:2 * D], W_qkv[:, 2 * D:3 * D]
    wout = np.ascontiguousarray(W_out)

    in_maps = []
    for c in range(NCORES):
        hA, hB = 2 * c, 2 * c + 1
        sl = np.s_[:, hA * DH:(hA + 1) * DH], np.s_[:, hB * DH:(hB + 1) * DH]
        m = {
            "xT": xT,
            "wq": np.ascontiguousarray(np.concatenate([Wq[sl[0]], Wq[sl[1]]], axis=1)),
            "wk": np.ascontiguousarray(np.concatenate([Wk[sl[0]], Wk[sl[1]]], axis=1)),
            "wv": np.ascontiguousarray(np.concatenate([Wv[sl[0]], Wv[sl[1]]], axis=1)),
            "cosT": cosT, "sinN": sinN, "ones16": ones16, "perm32": perm32,
            "wout": wout,
        }
        if has_bias:
            bq = np.stack([
                np.concatenate([b_qkv[hA * DH:(hA + 1) * DH], b_qkv[hB * DH:(hB + 1) * DH]]),
                np.concatenate([b_qkv[D + hA * DH:D + (hA + 1) * DH], b_qkv[D + hB * DH:D + (hB + 1) * DH]]),
                np.concatenate([b_qkv[2 * D + hA * DH:2 * D + (hA + 1) * DH], b_qkv[2 * D + hB * DH:2 * D + (hB + 1) * DH]]),
            ], axis=1).astype(np.float32)  # [128, 3]
            m["bqkv"] = bq
        in_maps.append(m)

    res = run_bass_kernel_spmd(nc, in_maps, core_ids=list(range(NCORES)))
    out = np.concatenate([res.results[c]["out"] for c in range(NCORES)], axis=0)
    out = out + b_out[None, :]
    return out.reshape(B, T, D).astype(np.float32)


if __name__ == "__main__":
    # quick self-check against a numpy reference
    rng = np.random.default_rng(0)
    x = rng.standard_normal((B, T, D)).astype(np.float32)
    mask = np.tril(np.ones((T, T), dtype=np.int32))[None, None]
    W_qkv = (rng.standard_normal((D, 3 * D)) * D ** -0.5).astype(np.float32)
    b_qkv = np.zeros(3 * D, np.float32)
    W_out = (rng.standard_normal((D, D)) * D ** -0.5).astype(np.float32)
    b_out = np.zeros(D, np.float32)

    def ref_np(x):
        q = x.reshape(TT, D) @ W_qkv[:, :D] + b_qkv[:D]
        k = x.reshape(TT, D) @ W_qkv[:, D:2 * D] + b_qkv[D:2 * D]
        v = x.reshape(TT, D) @ W_qkv[:, 2 * D:] + b_qkv[2 * D:]
        def heads(t):
            return t.reshape(B, T, H, DH).transpose(0, 2, 1, 3)
        q, k, v = heads(q), heads(k), heads(v)
        half = DH // 2
        idx = np.arange(half, dtype=np.float32)
        posn = np.arange(T, dtype=np.float32)[:, None]
        invf = 10000.0 ** (-2.0 * idx / (2.0 * half))
        ang = posn * invf
        cosv, sinv = np.cos(ang), np.sin(ang)
        def rot(t):
            a, b2 = t[..., :half], t[..., half:]
            return np.concatenate([a * cosv - b2 * sinv, a * sinv + b2 * cosv], -1)
        q, k = rot(q), rot(k)
        s = np.einsum("bhqd,bhkd->bhqk", q, k) / np.sqrt(DH)
        cm = np.tril(np.ones((T, T))) == 0
        s = np.where(cm[None, None], -np.inf, s)
        p = np.exp(s - s.max(-1, keepdims=True))
        p = p / p.sum(-1, keepdims=True)
        y = np.einsum("bhqk,bhkd->bhqd", p, v)
        y = y.transpose(0, 2, 1, 3).reshape(B, T, D)
        return y @ W_out + b_out

    got = kernel(x, mask, W_qkv, b_qkv, W_out, b_out)
    want = ref_np(x.astype(np.float64))
    err = np.abs(got - want).max()
    print(f"maxabs err vs np-f64 ref: {err:.3e}  (scale {np.abs(want).max():.3f})")


# revision 45
# speedup vs baseline: 1.0302x; 1.0274x over previous
"""Causal multi-head attention block (QKV proj + RoPE + causal softmax attention
+ output proj) for Trainium2, sharded over 8 NeuronCores.

Problem shapes (hardcoded): B=2, T=2048, DIM=1024, H=16, DH=64.

Sharding: tensor-parallel over heads. Core c owns heads {2c, 2c+1} for BOTH
batches: it computes Q/K/V projections for its 128 head-columns (reading the
full replicated x^T), runs RoPE + causal-softmax attention for its 4 (batch,
head) pairs, then an AllToAll redistributes y^T so each core holds all 1024
y-dims for a 512-token slice and computes that slice of y @ W_out.

All matmuls run as float32r (TF32-like rounded fp32, full PE rate at N>=512);
everything else fp32. Softmax skips the max-subtraction (scores are O(6) for
unit-scale inputs, exp is safe in fp32) and applies causality by multiplying
exp(scores) by a 0/1 triangular mask on the diagonal 128x512 blocks only;
strictly-future blocks are never computed.

b_qkv / b_out handling: b_qkv is structurally zero for this problem (spec fill
"zeros"); if a nonzero b_qkv is ever passed, a bias-enabled program variant is
built instead. b_out is added on the host.
"""

import numpy as np

B = 2
T = 2048
D = 1024
H = 16
DH = 64
NCORES = 8
TT = B * T  # 4096 tokens total
HPC = H // NCORES  # 2 heads per core

_CACHE = {}


def _build(with_collective=True, has_bias=False):
    """Build the SPMD Bass program. Returns (nc, in_names)."""
    import concourse.bass as bass
    import concourse.tile as tile
    from concourse import bacc, mybir, masks
    from contextlib import ExitStack

    F32 = mybir.dt.float32
    F32R = mybir.dt.float32r
    AF = mybir.ActivationFunctionType

    nc = bacc.Bacc("TRN2", target_bir_lowering=False, debug=False,
                   num_devices=NCORES if with_collective else 1)

    # ---- DRAM I/O ----------------------------------------------------------
    xT_d = nc.dram_tensor("xT", [D, TT], F32R, kind="ExternalInput").ap()
    wq_d = nc.dram_tensor("wq", [D, 128], F32R, kind="ExternalInput").ap()
    wk_d = nc.dram_tensor("wk", [D, 128], F32R, kind="ExternalInput").ap()
    wv_d = nc.dram_tensor("wv", [D, 128], F32R, kind="ExternalInput").ap()
    cos_d = nc.dram_tensor("cosT", [128, T], F32, kind="ExternalInput").ap()
    sin_d = nc.dram_tensor("sinN", [128, T], F32, kind="ExternalInput").ap()
    ones_d = nc.dram_tensor("ones16", [128, 16], F32R, kind="ExternalInput").ap()
    perm_d = nc.dram_tensor("perm32", [128, 128], F32R, kind="ExternalInput").ap()
    wout_d = nc.dram_tensor("wout", [D, D], F32R, kind="ExternalInput").ap()
    out_d = nc.dram_tensor("out", [TT // NCORES, D], F32, kind="ExternalOutput").ap()
    if has_bias:
        bq_d = nc.dram_tensor("bqkv", [128, 3], F32, kind="ExternalInput").ap()

    a2a_in = nc.dram_tensor("a2a_in", [NCORES, 128, TT // NCORES], F32R,
                            kind="Internal").ap()
    a2a_out = nc.dram_tensor("a2a_out", [NCORES, 128, TT // NCORES], F32R,
                             kind="Internal").ap()

    with tile.TileContext(nc) as tc:
        with ExitStack() as ctx:
            const = ctx.enter_context(tc.tile_pool(name="const", bufs=1))
            xtp = ctx.enter_context(tc.tile_pool(name="xtp", bufs=5))
            qkp = ctx.enter_context(tc.tile_pool(name="qkp", bufs=2))
            vtp = ctx.enter_context(tc.tile_pool(name="vtp", bufs=2))
            ybigp = ctx.enter_context(tc.tile_pool(name="ybigp", bufs=1))
            ropep = ctx.enter_context(tc.tile_pool(name="ropep", bufs=2))
            pp = ctx.enter_context(tc.tile_pool(name="pp", bufs=5))
            op = ctx.enter_context(tc.tile_pool(name="op", bufs=2))
            smallp = ctx.enter_context(tc.tile_pool(name="smallp", bufs=2))
            ps_s = ctx.enter_context(tc.tile_pool(name="ps_s", bufs=2, space="PSUM"))
            ps_y = ctx.enter_context(tc.tile_pool(name="ps_y", bufs=4, space="PSUM"))

            # ---- constants -------------------------------------------------
            wq_sb = const.tile([128, 1024], F32R, tag="wq")
            wk_sb = const.tile([128, 1024], F32R, tag="wk")
            wv_sb = const.tile([128, 1024], F32R, tag="wv")
            for wsb, wd in ((wq_sb, wq_d), (wk_sb, wk_d), (wv_sb, wv_d)):
                wdr = wd.rearrange("(c p) m -> p c m", p=128)
                nc.sync.dma_start(wsb[:], wdr[:])
            cos_sb = const.tile([128, T], F32, tag="cos")
            sin_sb = const.tile([128, T], F32, tag="sin")
            nc.sync.dma_start(cos_sb[:], cos_d[:])
            nc.sync.dma_start(sin_sb[:], sin_d[:])
            ident = const.tile([128, 128], F32, tag="ident")
            masks.make_identity(nc, ident[:])
            perm_sb = const.tile([128, 128], F32R, tag="perm")
            nc.sync.dma_start(perm_sb[:], perm_d[:])
            if has_bias:
                bq_sb = const.tile([128, 3], F32, tag="bq")
                nc.sync.dma_start(bq_sb[:], bq_d[:])

            ytile = ybigp.tile([128, TT], F32R, tag="ybig")  # y^T accumulator

            TC = T // 512  # 4 token chunks per batch

            QTs, KTs, vhs = [], [], []
            for b in range(B):
                # ---- QKV projection + RoPE for batch b ---------------------
                QT = qkp.tile([128, T], F32R, tag="qt")
                KT = qkp.tile([128, T], F32R, tag="kt")
                VT = qkp.tile([128, T], F32R, tag="vt", bufs=1)
                for tcx in range(TC):
                    col0 = b * T + tcx * 512
                    ps_qk = ps_s.tile([128, 1024], F32, tag="s")  # q | k
                    ps_v = ps_y.tile([128, 512], F32, tag="y", name="ps_v")
                    xTr = xT_d.rearrange("(c p) t -> p c t", p=128)
                    for kc4 in range(4):  # one DMA brings 2 contraction chunks
                        xt = xtp.tile([128, 2 * 512], F32R, tag="xt")
                        nc.sync.dma_start(
                            xt[:], xTr[:, 2 * kc4:2 * kc4 + 2, col0:col0 + 512])
                        for kcs in range(2):
                            kc = 2 * kc4 + kcs
                            st = (kc == 0)
                            sp = (kc == 7)
                            xts = xt[:, kcs * 512:(kcs + 1) * 512]
                            nc.tensor.matmul(ps_qk[:, 0:512], wq_sb[:, bass.ts(kc, 128)],
                                             xts, start=st, stop=sp)
                            nc.tensor.matmul(ps_qk[:, 512:1024], wk_sb[:, bass.ts(kc, 128)],
                                             xts, start=st, stop=sp)
                            nc.tensor.matmul(ps_v[:], wv_sb[:, bass.ts(kc, 128)],
                                             xts, start=st, stop=sp)
                    # V: plain copy psum -> VT (f32r), with b_v folded in if present
                    if has_bias:
                        nc.vector.tensor_scalar_add(
                            VT[:, tcx * 512:(tcx + 1) * 512], ps_v[:],
                            bq_sb[:, 2:3])
                    else:
                        nc.vector.tensor_copy(VT[:, tcx * 512:(tcx + 1) * 512],
                                              ps_v[:])
                    # RoPE for Q and K: copy psum -> sbuf (ACT, f32r), build the
                    # 32-block-swapped copy on the PE (perm matmul -> psum),
                    # then 4 muls + 2 adds on the DVE (tensor_tensor is 1x
                    # either way, so reading the swap from PSUM costs nothing).
                    qraw = ropep.tile([128, 1024], F32R, tag="qraw")
                    if has_bias:
                        nc.scalar.activation(qraw[:, 0:512], ps_qk[:, 0:512],
                                             AF.Identity, bias=bq_sb[:, 0:1])
                        nc.scalar.activation(qraw[:, 512:1024], ps_qk[:, 512:1024],
                                             AF.Identity, bias=bq_sb[:, 1:2])
                    else:
                        nc.scalar.activation(qraw[:], ps_qk[:], AF.Identity)
                    ps_sw = [ps_y.tile([128, 512], F32, tag="y", name=f"ps_sw{_i}")
                             for _i in range(2)]
                    for _i in range(2):
                        nc.tensor.matmul(ps_sw[_i][:], perm_sb[:],
                                         qraw[:, _i * 512:(_i + 1) * 512],
                                         start=True, stop=True)
                    tcols = slice(tcx * 512, (tcx + 1) * 512)
                    nc.gpsimd.tensor_mul(qraw[:, 0:512], qraw[:, 0:512], cos_sb[:, tcols])
                    nc.gpsimd.tensor_mul(qraw[:, 512:1024], qraw[:, 512:1024],
                                         cos_sb[:, tcols])
                    t2 = ropep.tile([128, 1024], F32, tag="t2")
                    nc.vector.tensor_mul(t2[:, 0:512], ps_sw[0][:], sin_sb[:, tcols])
                    nc.vector.tensor_mul(t2[:, 512:1024], ps_sw[1][:], sin_sb[:, tcols])
                    nc.vector.tensor_add(QT[:, tcols], qraw[:, 0:512], t2[:, 0:512])
                    nc.vector.tensor_add(KT[:, tcols], qraw[:, 512:1024], t2[:, 512:1024])

                # ---- V transpose into per-head augmented buffers -----------
                vh = [vtp.tile([128, 16 * 65], F32R, tag=f"vh{h}", name=f"vh{h}")
                      for h in range(2)]
                QTs.append(QT)
                KTs.append(KT)
                vhs.append(vh)
                for h in range(2):
                    ones_cols = vh[h][:].rearrange("p (n c) -> p n c", c=65)[:, :, 64]
                    nc.sync.dma_start(ones_cols, ones_d[:])
                for kb in range(16):
                    ps_t = ps_y.tile([128, 512], F32, tag="y", name="ps_t")
                    nc.tensor.transpose(ps_t[:, 0:128],
                                        VT[:, kb * 128:(kb + 1) * 128].bitcast(F32),
                                        ident[:])
                    for h in range(2):
                        nc.vector.tensor_copy(vh[h][:, kb * 65:kb * 65 + 64],
                                              ps_t[:, h * 64:h * 64 + 64])

            # ---- load W_out early: overlaps with attention compute ---------
            wout_sb = const.tile([128, 8 * 1024], F32R, tag="wout")
            wor = wout_d.rearrange("(c p) m -> p c m", p=128)
            nc.sync.dma_start(wout_sb[:], wor[:])

            for b in range(B):
                QT, KT, vh = QTs[b], KTs[b], vhs[b]
                # ---- attention for batch b ---------------------------------
                for qc in range(TC):
                    nkb = 4 * (qc + 1)  # causal: k chunks of 128 needed
                    qcols = slice(qc * 512, (qc + 1) * 512)
                    ps_yh = [ps_y.tile([128, 512], F32, tag="y", name=f"ps_y{_h}")
                             for _h in range(2)]
                    for kb in range(nkb):
                        # Diagonal k-chunks: q-columns < r are fully masked, so
                        # S / exp / PV all shrink to the valid span [r:512].
                        r = max(kb * 128 - qc * 512, 0)
                        nv = 512 - r  # valid q columns
                        ps_st = ps_s.tile([128, 1024], F32, tag="s")
                        for h in range(2):
                            nc.tensor.matmul(
                                ps_st[:, h * 512 + r:(h + 1) * 512],
                                KT[h * 64:(h + 1) * 64, kb * 128:(kb + 1) * 128],
                                QT[h * 64:(h + 1) * 64,
                                   qc * 512 + r:(qc + 1) * 512],
                                start=True, stop=True)
                        p = pp.tile([128, 1024], F32R, tag="p")
                        if r == 0:
                            nc.scalar.activation(p[:], ps_st[:], AF.Exp,
                                                 scale=float(DH) ** -0.5)
                        else:
                            for h in range(2):
                                nc.scalar.activation(
                                    p[:, h * 512 + r:(h + 1) * 512],
                                    ps_st[:, h * 512 + r:(h + 1) * 512],
                                    AF.Exp, scale=float(DH) ** -0.5)
                        if kb * 128 - qc * 512 >= 0:
                            # Zero strictly-future scores. Only the first 128
                            # valid columns can intersect the triangle (k-row
                            # p <= 127 < f for all later columns).
                            for h in range(2):
                                ph = p[:, h * 512 + r:h * 512 + r + 128]
                                nc.gpsimd.affine_select(
                                    out=ph, in_=ph, pattern=[[1, 128]],
                                    compare_op=mybir.AluOpType.is_ge,
                                    fill=0.0, base=0, channel_multiplier=-1)
                        for h in range(2):
                            nc.tensor.matmul(ps_yh[h][0:65, r:512],
                                             vh[h][:, kb * 65:(kb + 1) * 65],
                                             p[:, h * 512 + r:(h + 1) * 512],
                                             start=(kb == 0), stop=(kb == nkb - 1))
                    # normalize: y / denom (denom = psum row 64)
                    for h in range(2):
                        rr = smallp.tile([1, 512], F32, tag="r")
                        nc.vector.reciprocal(rr[:], ps_yh[h][64:65, :])
                        rb = smallp.tile([64, 512], F32, tag="rb")
                        nc.gpsimd.partition_broadcast(rb[:], rr[:], channels=64)
                        nc.vector.tensor_mul(
                            ytile[h * 64:(h + 1) * 64, b * T + qc * 512: b * T + (qc + 1) * 512],
                            ps_yh[h][0:64, :], rb[:])
                    jslice = 4 * b + qc
                    nc.sync.dma_start(a2a_in[jslice],
                                      ytile[:, jslice * 512:(jslice + 1) * 512])

            # ---- exchange y^T: all-to-all over token slices ----------------
            if with_collective:
                nc.gpsimd.collective_compute(
                    "AllToAll", mybir.AluOpType.bypass,
                    replica_groups=[list(range(NCORES))],
                    ins=[a2a_in[:]], outs=[a2a_out[:]])
            else:  # single-core timing/sim variant: fake the exchange
                for j in range(NCORES):
                    nc.sync.dma_start(a2a_out[j], a2a_in[j])
            yall = ybigp.tile([128, TT], F32R, tag="ybig")
            for i in range(NCORES):
                nc.sync.dma_start(yall[:, i * 512:(i + 1) * 512], a2a_out[i])

            # ---- output projection for my 512-token slice ------------------
            for tc4 in range(4):
                ps_o = ps_s.tile([128, 1024], F32, tag="s")
                for oc in range(2):
                    for i in range(8):
                        nc.tensor.matmul(
                            ps_o[:, oc * 512:(oc + 1) * 512],
                            yall[:, i * 512 + tc4 * 128: i * 512 + (tc4 + 1) * 128],
                            wout_sb[:, i * 1024 + oc * 512: i * 1024 + (oc + 1) * 512],
                            start=(i == 0), stop=(i == 7))
                o_sb = op.tile([128, 1024], F32, tag="o")
                nc.scalar.activation(o_sb[:], ps_o[:], AF.Identity)
                nc.sync.dma_start(out_d[tc4 * 128:(tc4 + 1) * 128, :], o_sb[:])

    nc.compile()
    return nc


def _host_tables():
    pos = np.arange(T, dtype=np.float32)
    idx = np.arange(32, dtype=np.float32)
    inv = (10000.0 ** (-2.0 * idx / 64.0)).astype(np.float32)
    ang = (inv[:, None] * pos[None, :]).astype(np.float32)  # [32, T]
    cos32 = np.cos(ang).astype(np.float32)
    sin32 = np.sin(ang).astype(np.float32)
    cosT = np.tile(cos32, (4, 1))  # [128, T]
    sinN = np.concatenate([-sin32, sin32, -sin32, sin32], axis=0)  # [128, T]
    # causal 0/1 masks for the 4 diagonal offsets r = 0,128,256,384
    ones16 = np.ones((128, 16), dtype=np.float32)
    perm32 = np.zeros((128, 128), dtype=np.float32)
    for j in range(128):
        blk, off = divmod(j, 32)
        perm32[(blk ^ 1) * 32 + off, j] = 1.0
    return cosT, sinN, ones16, perm32


def kernel(x, mask, W_qkv, b_qkv, W_out, b_out):
    from concourse.bass_utils import run_bass_kernel_spmd

    x = np.asarray(x, dtype=np.float32)
    W_qkv = np.asarray(W_qkv, dtype=np.float32)
    b_qkv = np.asarray(b_qkv, dtype=np.float32)
    W_out = np.asarray(W_out, dtype=np.float32)
    b_out = np.asarray(b_out, dtype=np.float32)

    has_bias = bool(np.any(b_qkv != 0.0))
    key = ("main", has_bias)
    if key not in _CACHE:
        _CACHE[key] = _build(with_collective=True, has_bias=has_bias)
    nc = _CACHE[key]

    X = x.reshape(TT, D)
    xT = np.ascontiguousarray(X.T)  # [D, TT]
    cosT, sinN, ones16, perm32 = _host_tables()
    Wq, Wk, Wv = W_qkv[:, 0:D], W_qkv[:, D:2 * D], W_qkv[:, 2 * D:3 * D]
    wout = np.ascontiguousarray(W_out)

    in_maps = []
    for c in range(NCORES):
        hA, hB = 2 * c, 2 * c + 1
        sl = np.s_[:, hA * DH:(hA + 1) * DH], np.s_[:, hB * DH:(hB + 1) * DH]
        m = {
            "xT": xT,
            "wq": np.ascontiguousarray(np.concatenate([Wq[sl[0]], Wq[sl[1]]], axis=1)),
            "wk": np.ascontiguousarray(np.concatenate([Wk[sl[0]], Wk[sl[1]]], axis=1)),
            "wv": np.ascontiguousarray(np.concatenate([Wv[sl[0]], Wv[sl[1]]], axis=1)),
            "cosT": cosT, "sinN": sinN, "ones16": ones16, "perm32": perm32,
            "wout": wout,
        }
        if has_bias:
            bq = np.stack([
                np.concatenate([b_qkv[hA * DH:(hA + 1) * DH], b_qkv[hB * DH:(hB + 1) * DH]]),
                np.concatenate([b_qkv[D + hA * DH:D + (hA + 1) * DH], b_qkv[D + hB * DH:D + (hB + 1) * DH]]),
                np.concatenate([b_qkv[2 * D + hA * DH:2 * D + (hA + 1) * DH], b_qkv[2 * D + hB * DH:2 * D + (hB + 1) * DH]]),
            ], axis=1).astype(np.float32)  # [128, 3]
            m["bqkv"] = bq
        in_maps.append(m)

    res = run_bass_kernel_spmd(nc, in_maps, core_ids=list(range(NCORES)))
    out = np.concatenate([res.results[c]["out"] for c in range(NCORES)], axis=0)
    out = out + b_out[None, :]
    return out.reshape(B, T, D).astype(np.float32)


if __name__ == "__main__":
    # quick self-check against a numpy reference
    rng = np.random.default_rng(0)
    x = rng.standard_normal((B, T, D)).astype(np.float32)
    mask = np.tril(np.ones((T, T), dtype=np.int32))[None, None]
    W_qkv = (rng.standard_normal((D, 3 * D)) * D ** -0.5).astype(np.float32)
    b_qkv = np.zeros(3 * D, np.float32)
    W_out = (rng.standard_normal((D, D)) * D ** -0.5).astype(np.float32)
    b_out = np.zeros(D, np.float32)

    def ref_np(x):
        q = x.reshape(TT, D) @ W_qkv[:, :D] + b_qkv[:D]
        k = x.reshape(TT, D) @ W_qkv[:, D:2 * D] + b_qkv[D:2 * D]
        v = x.reshape(TT, D) @ W_qkv[:, 2 * D:] + b_qkv[2 * D:]
        def heads(t):
            return t.reshape(B, T, H, DH).transpose(0, 2, 1, 3)
        q, k, v = heads(q), heads(k), heads(v)
        half = DH // 2
        idx = np.arange(half, dtype=np.float32)
        posn = np.arange(T, dtype=np.float32)[:, None]
        invf = 10000.0 ** (-2.0 * idx / (2.0 * half))
        ang = posn * invf
        cosv, sinv = np.cos(ang), np.sin(ang)
        def rot(t):
            a, b2 = t[..., :half], t[..., half:]
            return np.concatenate([a * cosv - b2 * sinv, a * sinv + b2 * cosv], -1)
        q, k = rot(q), rot(k)
        s = np.einsum("bhqd,bhkd->bhqk", q, k) / np.sqrt(DH)
        cm = np.tril(np.ones((T, T))) == 0
        s = np.where(cm[None, None], -np.inf, s)
        p = np.exp(s - s.max(-1, keepdims=True))
        p = p / p.sum(-1, keepdims=True)
        y = np.einsum("bhqk,bhkd->bhqd", p, v)
        y = y.transpose(0, 2, 1, 3).reshape(B, T, D)
        return y @ W_out + b_out

    got = kernel(x, mask, W_qkv, b_qkv, W_out, b_out)
    want = ref_np(x.astype(np.float64))
    err = np.abs(got - want).max()
    print(f"maxabs err vs np-f64 ref: {err:.3e}  (scale {np.abs(want).max():.3f})")
